# revision 5
# baseline (speedup 1.0000x reference)
# Contrastive loss (CLIP-style) on 8 Trainium2 NeuronCores.
#
# reference:
#   img = l2norm(image_embeds); txt = l2norm(text_embeds)        # [N, D]
#   sim = img @ txt.T                                            # [N, N]
#   loss = mean(logsumexp(sim - 1, axis=-1) - diag(sim))
#
# Distribution (per sharding hint): shard both embedding batches along N
# across the 8 cores. Each core:
#   1. l2-normalizes its own 1/8 row-block of img and txt (fp32), casts the
#      normalized blocks to bf16, writes them to DRAM scratch.
#   2. AllGathers the normalized bf16 text block -> full [N, D] bf16 text.
#   3. Streams column chunks of the gathered text through the DMA-transpose
#      xbar into [D, n] layout and runs the row-block GEMM on the PE
#      (bf16 x bf16 -> fp32 PSUM).
#   4. exp() on ScalarE with accum_out produces per-row partial sums of
#      exp(sim) directly; logsumexp and the positives (computed separately
#      as a fused row-dot of the core's own img/txt blocks) finish the
#      per-row values lse(sim)_i - pos_i.
# Host gathers the 8 x [1024] row values and returns mean - margin
# (logsumexp(sim - 1) == logsumexp(sim) - 1, and |sim| <= 1 so exp never
# overflows; no row-max pass is needed).

import os

import numpy as np

N_TOTAL = 8192
D_FULL = 1024
N_CORES = 8
P = 128
NCHUNK = 512
MARGIN = 1.0

LAST_EXEC_NS = None
LAST_PROFILE = None


def build_bass(n_total=N_TOTAL, d=D_FULL, n_cores=N_CORES, nchunk=NCHUNK):
    import concourse.mybir as mybir
    import concourse.tile as tile
    from concourse import bacc

    dt = mybir.dt
    Alu = mybir.AluOpType
    Act = mybir.ActivationFunctionType
    AxisX = mybir.AxisListType.X

    blk = n_total // n_cores
    kt = d // P  # contraction tiles
    mt = blk // P  # local row tiles
    g_n = n_total // nchunk  # column chunks
    assert blk % P == 0 and d % P == 0 and n_total % nchunk == 0
    assert nchunk % P == 0

    nc = bacc.Bacc(
        "TRN2", target_bir_lowering=False, debug=False, num_devices=n_cores
    )
    img = nc.dram_tensor("img_block", [blk, d], dt.float32, kind="ExternalInput")
    txt = nc.dram_tensor("txt_block", [blk, d], dt.float32, kind="ExternalInput")
    out = nc.dram_tensor("out_rows", [P, mt], dt.float32, kind="ExternalOutput")

    with tile.TileContext(nc) as tc:
        with (
            tc.tile_pool(name="dram", bufs=1, space="DRAM") as dram_pool,
            tc.tile_pool(name="persist", bufs=1) as persist,
            tc.tile_pool(name="nat", bufs=2) as nat,
            tc.tile_pool(name="small", bufs=2) as small,
            tc.tile_pool(name="txtTp", bufs=2) as txtTp,
            tc.tile_pool(name="expp", bufs=4) as expp,
            tc.tile_pool(name="psum", bufs=4, space="PSUM") as psum_pool,
        ):
            imgn_dram = dram_pool.tile([blk, d], dt.bfloat16, name="imgn_dram")
            txtn_dram = dram_pool.tile([blk, d], dt.bfloat16, name="txtn_dram")
            txt_ag = dram_pool.tile(
                [n_total, d], dt.bfloat16, name="txt_ag", addr_space="Shared"
            )

            pos_all = persist.tile([P, mt], dt.float32, name="pos_all")
            sums_all = persist.tile([P, mt * g_n], dt.float32, name="sums_all")
            out_all = persist.tile([P, mt], dt.float32, name="out_all")

            # ---- prologue A: normalize this core's txt block (gates the AG) ----
            inv_t_tiles = []
            txt_nat_tiles = []
            for t in range(mt):
                txt_nat = nat.tile([P, d], dt.float32, name="txt_nat", tag=f"txt_nat{t}")
                nc.sync.dma_start(txt_nat[:], txt[t * P : (t + 1) * P, :])
                tt_scr = nat.tile([P, d], dt.float32, name="tt_scr", tag="tt_scr")
                nc.scalar.activation(tt_scr[:], txt_nat[:], Act.Square)
                n2t = small.tile([P, 1], dt.float32, name="n2t", tag="n2t")
                nc.vector.reduce_sum(n2t[:], tt_scr[:], axis=AxisX)
                # 1/||x|| = sqrt(1/||x||^2)  (Rsqrt activation is banned)
                r2t = small.tile([P, 1], dt.float32, name="r2t", tag="r2t")
                nc.vector.reciprocal(r2t[:], n2t[:])
                invt = small.tile([P, 1], dt.float32, name=f"invt{t}", tag=f"invt{t}")
                nc.scalar.activation(invt[:], r2t[:], Act.Sqrt)
                txtn = nat.tile([P, d], dt.bfloat16, name="txtn", tag="txtn")
                nc.vector.tensor_scalar_mul(txtn[:], txt_nat[:], invt[:])
                nc.sync.dma_start(txtn_dram[t * P : (t + 1) * P, :], txtn[:])
                inv_t_tiles.append(invt)
                txt_nat_tiles.append(txt_nat)

            # ---- all-gather the normalized text blocks ----
            nc.gpsimd.collective_compute(
                "AllGather",
                Alu.bypass,
                replica_groups=[list(range(n_cores))],
                ins=[txtn_dram.opt()],
                outs=[txt_ag.opt()],
            )

            # ---- prologue B (overlaps the AG): img block + positives ----
            for t in range(mt):
                img_nat = nat.tile([P, d], dt.float32, name="img_nat", tag="img_nat")
                nc.sync.dma_start(img_nat[:], img[t * P : (t + 1) * P, :])
                sq_scr = nat.tile([P, d], dt.float32, name="sq_scr", tag="sq_scr")
                nc.scalar.activation(sq_scr[:], img_nat[:], Act.Square)
                n2i = small.tile([P, 1], dt.float32, name="n2i", tag="n2i")
                nc.vector.reduce_sum(n2i[:], sq_scr[:], axis=AxisX)
                r2i = small.tile([P, 1], dt.float32, name="r2i", tag="r2i")
                nc.vector.reciprocal(r2i[:], n2i[:])
                invi = small.tile([P, 1], dt.float32, name="invi", tag="invi")
                nc.scalar.activation(invi[:], r2i[:], Act.Sqrt)
                imgn = nat.tile([P, d], dt.bfloat16, name="imgn", tag="imgn")
                nc.scalar.mul(imgn[:], img_nat[:], invi[:])
                nc.sync.dma_start(imgn_dram[t * P : (t + 1) * P, :], imgn[:])

                # positives: raw row dot * inverse norms
                dot_scr = nat.tile([P, d], dt.float32, name="dot_scr", tag="dot_scr")
                nc.vector.tensor_mul(dot_scr[:], img_nat[:], txt_nat_tiles[t][:])
                dotv = small.tile([P, 1], dt.float32, name="dotv", tag="dotv")
                nc.vector.reduce_sum(dotv[:], dot_scr[:], axis=AxisX)
                pos_tmp = small.tile([P, 1], dt.float32, name="pos_tmp", tag="pos_tmp")
                nc.vector.tensor_scalar_mul(pos_tmp[:], dotv[:], invi[:])
                nc.vector.tensor_scalar_mul(
                    pos_all[:, t : t + 1], pos_tmp[:], inv_t_tiles[t][:]
                )

            # ---- img block in [D, M] layout via DMA-transpose ----
            imgT = []
            for k in range(kt):
                it = persist.tile([P, blk], dt.bfloat16, name=f"imgT{k}", tag=f"imgT{k}")
                nc.sync.dma_start(
                    it[:], imgn_dram[:, k * P : (k + 1) * P], transpose=True
                )
                imgT.append(it)

            # ---- main loop: row-block GEMM + exp row-sums ----
            for g in range(g_n):
                txtT = []
                for k in range(kt):
                    ttile = txtTp.tile(
                        [P, nchunk], dt.bfloat16, name=f"txtT{k}", tag=f"txtT{k}"
                    )
                    nc.sync.dma_start(
                        ttile[:],
                        txt_ag[g * nchunk : (g + 1) * nchunk, k * P : (k + 1) * P],
                        transpose=True,
                    )
                    txtT.append(ttile)
                for m in range(mt):
                    ps = psum_pool.tile([P, nchunk], dt.float32, name="ps", tag="ps")
                    for k in range(kt):
                        nc.tensor.matmul(
                            ps[:],
                            lhsT=imgT[k][:, m * P : (m + 1) * P],
                            rhs=txtT[k][:],
                            start=(k == 0),
                            stop=(k == kt - 1),
                        )
                    ex = expp.tile([P, nchunk], dt.float32, name="ex", tag="ex")
                    idx = m * g_n + g
                    nc.scalar.activation(
                        ex[:], ps[:], Act.Exp, accum_out=sums_all[:, idx : idx + 1]
                    )

            # ---- tail: lse - positives per local row ----
            for m in range(mt):
                rs = small.tile([P, 1], dt.float32, name="rs", tag="rs")
                nc.vector.reduce_sum(
                    rs[:], sums_all[:, m * g_n : (m + 1) * g_n], axis=AxisX
                )
                lse = small.tile([P, 1], dt.float32, name="lse", tag="lse")
                nc.scalar.activation(lse[:], rs[:], Act.Ln)
                nc.vector.tensor_scalar_sub(
                    out_all[:, m : m + 1], lse[:], pos_all[:, m : m + 1]
                )

            nc.sync.dma_start(out.ap(), out_all[:])

    nc.compile()
    return nc


_NC_CACHE = {}


def _get_nc(key=(N_TOTAL, D_FULL, N_CORES, NCHUNK)):
    if key not in _NC_CACHE:
        _NC_CACHE[key] = build_bass(*key)
    return _NC_CACHE[key]


def kernel(image_embeds: np.ndarray, text_embeds: np.ndarray) -> np.ndarray:
    global LAST_EXEC_NS, LAST_PROFILE
    from concourse import bass_utils

    image_embeds = np.ascontiguousarray(np.asarray(image_embeds, dtype=np.float32))
    text_embeds = np.ascontiguousarray(np.asarray(text_embeds, dtype=np.float32))
    assert image_embeds.shape == (N_TOTAL, D_FULL)
    assert text_embeds.shape == (N_TOTAL, D_FULL)

    nc = _get_nc()
    blk = N_TOTAL // N_CORES
    in_maps = [
        {
            "img_block": np.ascontiguousarray(image_embeds[c * blk : (c + 1) * blk]),
            "txt_block": np.ascontiguousarray(text_embeds[c * blk : (c + 1) * blk]),
        }
        for c in range(N_CORES)
    ]
    trace = os.environ.get("KERNEL_TRACE", "0") == "1"
    res = bass_utils.run_bass_kernel_spmd(
        nc, in_maps, core_ids=list(range(N_CORES)), trace=trace
    )
    LAST_EXEC_NS = res.exec_time_ns
    LAST_PROFILE = res.profile_json
    globals()["LAST_RESULT"] = res

    mt = blk // P
    rows = []
    for c in range(N_CORES):
        o = np.asarray(res.results[c]["out_rows"])  # [P, mt]
        rows.append(o.T.reshape(-1))  # local row i = m*P + p
    vals = np.concatenate(rows)  # [N_TOTAL]
    result = np.float32(np.mean(vals.astype(np.float64)) - MARGIN)
    return np.asarray(result, dtype=np.float32)
